# revision 8
# baseline (speedup 1.0000x reference)
"""Trainium2 Bass kernel for nn_AttnMatching.

Reference computes:
    emb = emb_table[1:L+1]                      # [L, D]
    attn = einsum('ld,ntd->nlt', emb, self_attn)
    out  = einsum('nlt,t->nl', attn, value_w[0])

Reassociated (identical math):
    ctx[n, d] = sum_t value_w[t] * self_attn[n, t, d]    # [N, D]  (tiny:
              #  0.1% of total FLOPs -> folded on host during marshalling)
    out[n, l] = sum_d ctx[n, d] * emb[l, d]              # [N, L]

Memory-bound: dominant traffic is streaming the embedding table. All
device traffic is bf16 (rel_norm vs fp32 reference ~3e-3, gate is 2e-2):
the emb shard halves to 1.6 MB/core and the PE streams bf16 ~3x faster
than fp32.

Sharding: vocab axis L split across 8 cores (6250 cols each), no
communication. Host-side marshalling per core: the ctxT [D,16] block and
each DMA chunk of the emb shard are separate contiguous DRAM tensors
(sequential HBM reads, 4 KB packets); outputs are per-store-group
contiguous DRAM tensors, concatenated and upcast on host.

Per-core program (raw bacc, hand-rolled sems):
  - chunk loads issued in the entry block, alternating sync/scalar HWDGE
    rings so chunks complete in need-order (SDMA round-robins rings at
    packet granularity; the next-needed chunk is always at the head of
    the other ring). Chunk 0 (ctxT + one matmul of cols) is small so
    compute starts early.
  - PE: dependency-free bf16 warmup matmuls on scratch (uninitialized --
    only timing matters) bridge the HAM clock-gate window until data
    lands, then mains: lhsT = ctxT [128,16], rhs = emb cols [128,<=512]
    -> PSUM [16,<=512], 8-bank rotation.
  - PSUM -> SBUF bf16 copies alternate between DVE (even) and ACT (odd);
    the two engines touch disjoint PSUM banks.
  - stores: per-group [16, cols] bf16 DMAs on the gpsimd SWDGE ring
    (otherwise idle), gated on the copy sems. No completion wait: the
    epilogue's clear_and_free dma_reset drains the store queue before
    the NEFF can complete.
  - Epilogue: sem-only all-engine barrier + semaphore clear so the NEFF
    is safe to re-execute.
"""

import os

import numpy as np
import ml_dtypes

L = 50000
D = 128
T = 100
N = 16
NCORES = 8
LSH = L // NCORES          # 6250 columns per core
CTX = 16                   # ctxT [D, N] prepended to chunk 0
MM = 512                   # matmul moving-operand / PSUM bank limit

# knobs (env-overridable for A/B profiling)
DMA_CHUNK = int(os.environ.get("K_DMA_CHUNK", "1024"))  # emb load cols/chunk
N_WARMUP = int(os.environ.get("K_N_WARMUP", "5"))       # PE HAM warmup matmuls
NPS = int(os.environ.get("K_NPS", "8"))                 # PSUM banks in rotation
# per-chunk ring pattern, cycled: scalar pays a ~1.3us ACT-table load in
# its entry stream, so it only gets the last-needed chunk
LOAD_RINGS = os.environ.get("K_LOAD_RINGS", "sgsgsga")
STORE_RINGS = os.environ.get("K_STORE_RINGS", "as")     # ring cycle for stores
# store groups as mm-index boundaries; last group tiny so the final
# store (the tail) is short
STORE_BOUNDS = [
    int(x) for x in os.environ.get("K_STORE_BOUNDS", "0,5,9,12,13").split(",")
]
NUM_DEVICES = int(os.environ.get("K_NUM_DEVICES", str(NCORES)))

_cache = {}


def _chunks(total, step, start=0):
    return [(c0, min(c0 + step, total)) for c0 in range(start, total, step)]


def _plan():
    # chunk i covers emb cols [a, b); chunk 0 also carries ctxT
    ch = [(0, MM)] + _chunks(LSH, DMA_CHUNK, start=MM)
    mm_cols = _chunks(LSH, MM)
    gates = []
    for _c0, c1 in mm_cols:
        gates.append(next(i for i, (_a, b) in enumerate(ch) if b >= c1))
    n_mm = len(mm_cols)
    bounds = [min(b, n_mm) for b in STORE_BOUNDS]
    assert bounds[0] == 0 and bounds[-1] == n_mm, bounds
    store_groups = [
        (bounds[g], bounds[g + 1])
        for g in range(len(bounds) - 1)
        if bounds[g + 1] > bounds[g]
    ]
    return ch, mm_cols, gates, store_groups


def _build():
    import concourse.bacc as bacc
    import concourse.mybir as mybir

    f32 = mybir.dt.float32
    bf16 = mybir.dt.bfloat16

    nc = bacc.Bacc(
        "TRN2",
        target_bir_lowering=False,
        debug=False,
        enable_asserts=True,
        num_devices=NUM_DEVICES,
    )

    ch, mm_cols, gates, store_groups = _plan()
    nch = len(ch)
    n_mm = len(mm_cols)

    # one contiguous DRAM tensor per load chunk / store group
    emb_t = []
    for i, (a, b) in enumerate(ch):
        cols = (CTX if i == 0 else 0) + (b - a)
        emb_t.append(
            nc.dram_tensor(f"emb{i}", [D, cols], bf16, kind="ExternalInput").ap()
        )
    out_t = []
    for g, (m0, m1) in enumerate(store_groups):
        cols = mm_cols[m1 - 1][1] - mm_cols[m0][0]
        out_t.append(
            nc.dram_tensor(f"out{g}", [N, cols], bf16, kind="ExternalOutput").ap()
        )

    embx_sb = nc.alloc_sbuf_tensor("embx_sb", [D, CTX + LSH], bf16).ap()
    out_sb = nc.alloc_sbuf_tensor("out_sb", [N, LSH], bf16).ap()
    wscr = nc.alloc_sbuf_tensor("wscr", [D, CTX + MM], bf16).ap()
    ps = [
        nc.alloc_psum_tensor(f"ps{j}", [N, MM], f32).ap() for j in range(NPS)
    ]

    lde = [nc.alloc_semaphore(f"lde{i}") for i in range(nch)]
    mm_sem = nc.alloc_semaphore("mm")
    cpv = nc.alloc_semaphore("cpv")
    cpa = nc.alloc_semaphore("cpa")
    st = nc.alloc_semaphore("st")
    all_sems = lde + [mm_sem, cpv, cpa, st]

    eng = {"s": nc.sync, "a": nc.scalar, "g": nc.gpsimd}

    # entry block: all chunk loads, alternating rings, need-order heads
    for i, (a, b) in enumerate(ch):
        ring = eng[LOAD_RINGS[i % len(LOAD_RINGS)]]
        s0 = 0 if i == 0 else CTX + a
        s1 = CTX + b
        ring.dma_start(embx_sb[:, s0:s1], emb_t[i][:, :]).then_inc(lde[i], 16)

    with nc.Block() as block:

        @block.tensor
        def _(t):
            for _wi in range(N_WARMUP):
                nc.tensor.matmul(
                    ps[NPS - 1][:, :],
                    lhsT=wscr[:, :CTX],
                    rhs=wscr[:, CTX:],
                    start=True,
                    stop=True,
                )
            prev_gate = -1
            for s, (c0, c1) in enumerate(mm_cols):
                if gates[s] != prev_gate:
                    t.wait_ge(lde[gates[s]], 16)
                    prev_gate = gates[s]
                if s >= NPS:
                    prev = s - NPS
                    if prev % 2 == 0:
                        t.wait_ge(cpv, prev // 2 + 1)
                    else:
                        t.wait_ge(cpa, prev // 2 + 1)
                nc.tensor.matmul(
                    ps[s % NPS][:, : c1 - c0],
                    lhsT=embx_sb[:, :CTX],
                    rhs=embx_sb[:, CTX + c0 : CTX + c1],
                    start=True,
                    stop=True,
                ).then_inc(mm_sem, 1)

        @block.vector
        def _(v):
            for s in range(0, n_mm, 2):
                v.wait_ge(mm_sem, s + 1)
                c0, c1 = mm_cols[s]
                nc.vector.tensor_copy(
                    out_sb[:, c0:c1], ps[s % NPS][:, : c1 - c0]
                ).then_inc(cpv, 1)

        # store group g needs every copy with mm-index < m1:
        #   cpv >= ceil(m1/2) (DVE, even mms), cpa >= m1//2 (ACT, odd mms)
        # groups ride the HWDGE rings per STORE_RINGS; scalar-ring stores
        # are interleaved into the ACT copy stream right after ACT's own
        # contribution completes, sync-ring stores issue from the (idle
        # after loads) sync engine.
        store_rings = [
            STORE_RINGS[g % len(STORE_RINGS)] for g in range(len(store_groups))
        ]

        def _store(issuer, g):
            m0, m1 = store_groups[g]
            n_even = (m1 + 1) // 2
            if n_even:
                issuer.wait_ge(cpv, n_even)
            n_odd = m1 // 2
            if n_odd and store_rings[g] != "a":
                issuer.wait_ge(cpa, n_odd)
            a = mm_cols[m0][0]
            b = mm_cols[m1 - 1][1]
            eng[store_rings[g]].dma_start(
                out_t[g][:, :], out_sb[:, a:b]
            ).then_inc(st, 16)

        @block.scalar
        def _(sc):
            act_stores = {}  # after ACT copy s, issue these groups
            for g, (m0, m1) in enumerate(store_groups):
                if store_rings[g] == "a":
                    n_odd = m1 // 2
                    act_stores.setdefault(
                        2 * n_odd - 1 if n_odd else -1, []
                    ).append(g)
            for g in act_stores.get(-1, []):
                _store(sc, g)
            for s in range(1, n_mm, 2):
                sc.wait_ge(mm_sem, s + 1)
                c0, c1 = mm_cols[s]
                nc.scalar.copy(
                    out_sb[:, c0:c1], ps[s % NPS][:, : c1 - c0]
                ).then_inc(cpa, 1)
                for g in act_stores.get(s, []):
                    _store(sc, g)

        @block.sync
        def _(sy):
            for g in range(len(store_groups)):
                if store_rings[g] == "s":
                    _store(sy, g)

    # epilogue: the Block exit already emitted a full all-engine barrier,
    # so go straight to the sem clear (re-execution safety); the clear's
    # dma_reset drains the store queue so the final writes land.
    nc.clear_and_free_semaphores(all_sems)

    nc.compile()
    return nc


def _get_nc():
    if "nc" not in _cache:
        _cache["nc"] = _build()
    return _cache["nc"]


def _make_in_maps(self_attn, emb_table, value_w):
    bf = ml_dtypes.bfloat16
    sa = np.asarray(self_attn, dtype=np.float32)
    w = np.asarray(value_w, dtype=np.float32)[0]
    ctxT = np.einsum("ntd,t->dn", sa, w).astype(bf)          # [D, N]
    embT = np.asarray(emb_table, dtype=np.float32)[1 : L + 1].T.astype(bf)
    ch, _mm_cols, _gates, _sg = _plan()
    maps = []
    for k in range(NCORES):
        shard = embT[:, k * LSH : (k + 1) * LSH]
        m = {}
        for i, (a, b) in enumerate(ch):
            if i == 0:
                blk = np.concatenate([ctxT, shard[:, a:b]], axis=1)
            else:
                blk = shard[:, a:b]
            m[f"emb{i}"] = np.ascontiguousarray(blk)
        maps.append(m)
    return maps


def run(self_attn, emb_table, value_w, trace=False):
    from concourse.bass_utils import run_bass_kernel_spmd

    nc = _get_nc()
    in_maps = _make_in_maps(self_attn, emb_table, value_w)
    res = run_bass_kernel_spmd(nc, in_maps, list(range(NCORES)), trace=trace)
    _ch, _mm, _g, store_groups = _plan()
    full = np.concatenate(
        [
            np.concatenate(
                [np.asarray(res.results[k][f"out{g}"]) for g in range(len(store_groups))],
                axis=1,
            )
            for k in range(NCORES)
        ],
        axis=1,
    ).astype(np.float32)
    return full, res


def kernel(self_attn, mat2, traj, emb_table, value_w):
    full, _ = run(self_attn, emb_table, value_w, trace=False)
    return full


# revision 9
# speedup vs baseline: 1.2315x; 1.2315x over previous
"""Trainium2 Bass kernel for nn_AttnMatching.

Reference computes:
    emb = emb_table[1:L+1]                      # [L, D]
    attn = einsum('ld,ntd->nlt', emb, self_attn)
    out  = einsum('nlt,t->nl', attn, value_w[0])

Reassociated (identical math):
    ctx[n, d] = sum_t value_w[t] * self_attn[n, t, d]    # [N, D]  (tiny:
              #  0.1% of total FLOPs -> folded on host during marshalling)
    out[n, l] = sum_d ctx[n, d] * emb[l, d]              # [N, L]

Memory-bound: dominant traffic is streaming the embedding table. All
device traffic is bf16 (rel_norm vs fp32 reference ~3e-3, gate is 2e-2):
the emb shard halves to 1.6 MB/core and the PE streams bf16 ~3x faster
than fp32.

Sharding: vocab axis L split across 8 cores (6250 cols each), no
communication. Host-side marshalling per core: the ctxT [D,16] block and
each DMA chunk of the emb shard are separate contiguous DRAM tensors
(sequential HBM reads, 4 KB packets); outputs are per-store-group
contiguous DRAM tensors, concatenated and upcast on host.

Per-core program (raw bacc, hand-rolled sems):
  - chunk loads issued in the entry block, alternating sync/scalar HWDGE
    rings so chunks complete in need-order (SDMA round-robins rings at
    packet granularity; the next-needed chunk is always at the head of
    the other ring). Chunk 0 (ctxT + one matmul of cols) is small so
    compute starts early.
  - PE: dependency-free bf16 warmup matmuls on scratch (uninitialized --
    only timing matters) bridge the HAM clock-gate window until data
    lands, then mains: lhsT = ctxT [128,16], rhs = emb cols [128,<=512]
    -> PSUM [16,<=512], 8-bank rotation.
  - PSUM -> SBUF bf16 copies alternate between DVE (even) and ACT (odd);
    the two engines touch disjoint PSUM banks.
  - stores: per-group [16, cols] bf16 DMAs on the gpsimd SWDGE ring
    (otherwise idle), gated on the copy sems. No completion wait: the
    epilogue's clear_and_free dma_reset drains the store queue before
    the NEFF can complete.
  - Epilogue: sem-only all-engine barrier + semaphore clear so the NEFF
    is safe to re-execute.
"""

import os

import numpy as np
import ml_dtypes

L = 50000
D = 128
T = 100
N = 16
NCORES = 8
LSH = L // NCORES          # 6250 columns per core
CTX = 16                   # ctxT [D, N] prepended to chunk 0
MM = 512                   # matmul moving-operand / PSUM bank limit

# knobs (env-overridable for A/B profiling)
DMA_CHUNK = int(os.environ.get("K_DMA_CHUNK", "1024"))  # emb load cols/chunk
N_WARMUP = int(os.environ.get("K_N_WARMUP", "8"))       # PE HAM warmup matmuls
NPS = int(os.environ.get("K_NPS", "8"))                 # PSUM banks in rotation
# per-chunk ring pattern, cycled: scalar pays a ~1.3us ACT-table load in
# its entry stream, so it only gets the last-needed chunk
LOAD_RINGS = os.environ.get("K_LOAD_RINGS", "sag")
STORE_RINGS = os.environ.get("K_STORE_RINGS", "as")     # ring cycle for stores
# store groups as mm-index boundaries; last group tiny so the final
# store (the tail) is short
STORE_BOUNDS = [
    int(x) for x in os.environ.get("K_STORE_BOUNDS", "0,5,9,12,13").split(",")
]
NUM_DEVICES = int(os.environ.get("K_NUM_DEVICES", str(NCORES)))

_cache = {}


def _chunks(total, step, start=0):
    return [(c0, min(c0 + step, total)) for c0 in range(start, total, step)]


def _plan():
    # chunk i covers emb cols [a, b); chunk 0 also carries ctxT
    ch = [(0, MM)] + _chunks(LSH, DMA_CHUNK, start=MM)
    mm_cols = _chunks(LSH, MM)
    gates = []
    for _c0, c1 in mm_cols:
        gates.append(next(i for i, (_a, b) in enumerate(ch) if b >= c1))
    n_mm = len(mm_cols)
    bounds = [min(b, n_mm) for b in STORE_BOUNDS]
    assert bounds[0] == 0 and bounds[-1] == n_mm, bounds
    store_groups = [
        (bounds[g], bounds[g + 1])
        for g in range(len(bounds) - 1)
        if bounds[g + 1] > bounds[g]
    ]
    return ch, mm_cols, gates, store_groups


def _build():
    import concourse.bacc as bacc
    import concourse.mybir as mybir

    f32 = mybir.dt.float32
    bf16 = mybir.dt.bfloat16

    nc = bacc.Bacc(
        "TRN2",
        target_bir_lowering=False,
        debug=False,
        enable_asserts=True,
        num_devices=NUM_DEVICES,
    )

    ch, mm_cols, gates, store_groups = _plan()
    nch = len(ch)
    n_mm = len(mm_cols)

    # one contiguous DRAM tensor per load chunk / store group
    emb_t = []
    for i, (a, b) in enumerate(ch):
        cols = (CTX if i == 0 else 0) + (b - a)
        emb_t.append(
            nc.dram_tensor(f"emb{i}", [D, cols], bf16, kind="ExternalInput").ap()
        )
    out_t = []
    for g, (m0, m1) in enumerate(store_groups):
        cols = mm_cols[m1 - 1][1] - mm_cols[m0][0]
        out_t.append(
            nc.dram_tensor(f"out{g}", [N, cols], bf16, kind="ExternalOutput").ap()
        )

    embx_sb = nc.alloc_sbuf_tensor("embx_sb", [D, CTX + LSH], bf16).ap()
    out_sb = nc.alloc_sbuf_tensor("out_sb", [N, LSH], bf16).ap()
    wscr = nc.alloc_sbuf_tensor("wscr", [D, CTX + MM], bf16).ap()
    ps = [
        nc.alloc_psum_tensor(f"ps{j}", [N, MM], f32).ap() for j in range(NPS)
    ]

    lde = [nc.alloc_semaphore(f"lde{i}") for i in range(nch)]
    mm_sem = nc.alloc_semaphore("mm")
    cpv = nc.alloc_semaphore("cpv")
    cpa = nc.alloc_semaphore("cpa")
    st = nc.alloc_semaphore("st")
    all_sems = lde + [mm_sem, cpv, cpa, st]

    eng = {"s": nc.sync, "a": nc.scalar, "g": nc.gpsimd}

    # entry block: all chunk loads, alternating rings, need-order heads
    for i, (a, b) in enumerate(ch):
        ring = eng[LOAD_RINGS[i % len(LOAD_RINGS)]]
        s0 = 0 if i == 0 else CTX + a
        s1 = CTX + b
        ring.dma_start(embx_sb[:, s0:s1], emb_t[i][:, :]).then_inc(lde[i], 16)

    with nc.Block() as block:

        @block.tensor
        def _(t):
            for _wi in range(N_WARMUP):
                nc.tensor.matmul(
                    ps[NPS - 1][:, :],
                    lhsT=wscr[:, :CTX],
                    rhs=wscr[:, CTX:],
                    start=True,
                    stop=True,
                )
            prev_gate = -1
            for s, (c0, c1) in enumerate(mm_cols):
                if gates[s] != prev_gate:
                    t.wait_ge(lde[gates[s]], 16)
                    prev_gate = gates[s]
                if s >= NPS:
                    prev = s - NPS
                    if prev % 2 == 0:
                        t.wait_ge(cpv, prev // 2 + 1)
                    else:
                        t.wait_ge(cpa, prev // 2 + 1)
                nc.tensor.matmul(
                    ps[s % NPS][:, : c1 - c0],
                    lhsT=embx_sb[:, :CTX],
                    rhs=embx_sb[:, CTX + c0 : CTX + c1],
                    start=True,
                    stop=True,
                ).then_inc(mm_sem, 1)

        @block.vector
        def _(v):
            for s in range(0, n_mm, 2):
                v.wait_ge(mm_sem, s + 1)
                c0, c1 = mm_cols[s]
                nc.vector.tensor_copy(
                    out_sb[:, c0:c1], ps[s % NPS][:, : c1 - c0]
                ).then_inc(cpv, 1)

        # store group g needs every copy with mm-index < m1:
        #   cpv >= ceil(m1/2) (DVE, even mms), cpa >= m1//2 (ACT, odd mms)
        # groups ride the HWDGE rings per STORE_RINGS; scalar-ring stores
        # are interleaved into the ACT copy stream right after ACT's own
        # contribution completes, sync-ring stores issue from the (idle
        # after loads) sync engine.
        store_rings = [
            STORE_RINGS[g % len(STORE_RINGS)] for g in range(len(store_groups))
        ]

        def _store(issuer, g):
            m0, m1 = store_groups[g]
            n_even = (m1 + 1) // 2
            if n_even:
                issuer.wait_ge(cpv, n_even)
            n_odd = m1 // 2
            if n_odd and store_rings[g] != "a":
                issuer.wait_ge(cpa, n_odd)
            a = mm_cols[m0][0]
            b = mm_cols[m1 - 1][1]
            eng[store_rings[g]].dma_start(
                out_t[g][:, :], out_sb[:, a:b]
            ).then_inc(st, 16)

        @block.scalar
        def _(sc):
            act_stores = {}  # after ACT copy s, issue these groups
            for g, (m0, m1) in enumerate(store_groups):
                if store_rings[g] == "a":
                    n_odd = m1 // 2
                    act_stores.setdefault(
                        2 * n_odd - 1 if n_odd else -1, []
                    ).append(g)
            for g in act_stores.get(-1, []):
                _store(sc, g)
            for s in range(1, n_mm, 2):
                sc.wait_ge(mm_sem, s + 1)
                c0, c1 = mm_cols[s]
                nc.scalar.copy(
                    out_sb[:, c0:c1], ps[s % NPS][:, : c1 - c0]
                ).then_inc(cpa, 1)
                for g in act_stores.get(s, []):
                    _store(sc, g)

        @block.sync
        def _(sy):
            for g in range(len(store_groups)):
                if store_rings[g] == "s":
                    _store(sy, g)

    # epilogue: the Block exit already emitted a full all-engine barrier,
    # so go straight to the sem clear (re-execution safety); the clear's
    # dma_reset drains the store queue so the final writes land.
    nc.clear_and_free_semaphores(all_sems)

    nc.compile()
    return nc


def _get_nc():
    if "nc" not in _cache:
        _cache["nc"] = _build()
    return _cache["nc"]


def _make_in_maps(self_attn, emb_table, value_w):
    bf = ml_dtypes.bfloat16
    sa = np.asarray(self_attn, dtype=np.float32)
    w = np.asarray(value_w, dtype=np.float32)[0]
    ctxT = np.einsum("ntd,t->dn", sa, w).astype(bf)          # [D, N]
    embT = np.asarray(emb_table, dtype=np.float32)[1 : L + 1].T.astype(bf)
    ch, _mm_cols, _gates, _sg = _plan()
    maps = []
    for k in range(NCORES):
        shard = embT[:, k * LSH : (k + 1) * LSH]
        m = {}
        for i, (a, b) in enumerate(ch):
            if i == 0:
                blk = np.concatenate([ctxT, shard[:, a:b]], axis=1)
            else:
                blk = shard[:, a:b]
            m[f"emb{i}"] = np.ascontiguousarray(blk)
        maps.append(m)
    return maps


def run(self_attn, emb_table, value_w, trace=False):
    from concourse.bass_utils import run_bass_kernel_spmd

    nc = _get_nc()
    in_maps = _make_in_maps(self_attn, emb_table, value_w)
    res = run_bass_kernel_spmd(nc, in_maps, list(range(NCORES)), trace=trace)
    _ch, _mm, _g, store_groups = _plan()
    full = np.concatenate(
        [
            np.concatenate(
                [np.asarray(res.results[k][f"out{g}"]) for g in range(len(store_groups))],
                axis=1,
            )
            for k in range(NCORES)
        ],
        axis=1,
    ).astype(np.float32)
    return full, res


def kernel(self_attn, mat2, traj, emb_table, value_w):
    full, _ = run(self_attn, emb_table, value_w, trace=False)
    return full
